# revision 5
# baseline (speedup 1.0000x reference)
"""GaussianNB log-posterior kernel for 8 Trainium2 NeuronCores.

out[b, c] = log_pi[c] - 0.5 * sum_f(log2pi + log_var[c,f] + (x[b,f]-mu[c,f])^2 / var[c,f])
          = const_c + sum_f x^2[b,f]*wq[c,f] + sum_f x[b,f]*wc[c,f]
  with wq = -0.5*exp(-lv), wc = mu*exp(-lv),
       const_c = lp_c - 0.5*(F*log2pi + sum_f lv + sum_f mu^2*exp(-lv))

Strategy: data-parallel over batch (B=2048 -> 256 rows/core), weights replicated.
Wire format fp16, f-major (host does layout only: cast + transpose + pack;
all arithmetic on device). Per core:
  - DMA lv, x, mu (f-major packed (128, 8*256) fp16), pipelined in that order
  - ACT: inv = exp(-lv); DVE: wq, wc, x^2, mu^2*inv, t = lv + mu^2*inv
  - PE: out[b,c] via 32 fp16 matmuls (stationary = x/x^2 b-chunks, moving = w
    tiles); const_c folded into the same PSUM accumulation with a K=1
    ones-row matmul; sum_f reductions via ones-column stationary matmuls.
  - PE warmup matmuls during the DMA window defeat the p-state ramp.
Output (256b, 256c) fp16, host casts to f32.
"""
import sys

sys.path.insert(0, "/opt/trn_rl_repo")
import numpy as np
import concourse.bacc as bacc
import concourse.mybir as mybir
from concourse.tile import TileContext
from concourse.bass_utils import run_bass_kernel_spmd

B, C, F = 2048, 256, 1024
NCORES = 8
BSH = B // NCORES  # 256
KT = F // 128      # 8 k-tiles
LOG_2PI = float(np.log(2.0 * np.pi))
F32 = mybir.dt.float32
F16 = mybir.dt.float16
OP = mybir.AluOpType
AF = mybir.ActivationFunctionType
NWARM = 26

_CACHE = {}


def _build():
    nc = bacc.Bacc("TRN2", target_bir_lowering=False, debug=False, num_devices=NCORES)
    # f-major packed inputs: [:, k*256:(k+1)*256] = rows k*128..(k+1)*128 of the
    # (F, ...) transposed tensor. mu carries lp as a (1, 256) row on partition 0.
    lv_d = nc.dram_tensor("lvt", [128, 2 * F], F16, kind="ExternalInput").ap()
    x_d = nc.dram_tensor("xt", [128, 2 * F], F16, kind="ExternalInput").ap()
    mu_d = nc.dram_tensor("mut", [128, 2 * F + 256], F16, kind="ExternalInput").ap()
    out_d = nc.dram_tensor("out", [128, 2 * BSH], F16, kind="ExternalOutput").ap()

    with TileContext(nc) as tc:
        with (
            tc.tile_pool(name="sb", bufs=1) as sb,
            tc.tile_pool(name="po", bufs=1, space="PSUM") as po,
        ):
            lvt = sb.tile([128, 2 * F], F16, tag="lvt")
            xt = sb.tile([128, 2 * F], F16, tag="xt")
            mut = sb.tile([128, 2 * F + 256], F16, tag="mut")
            # halves (4 k-tiles = 1024 cols each) for DMA/compute pipelining
            for m in range(2):
                nc.sync.dma_start(out=lvt[:, m * F:(m + 1) * F], in_=lv_d[:, m * F:(m + 1) * F])
            for m in range(2):
                nc.sync.dma_start(out=xt[:, m * F:(m + 1) * F], in_=x_d[:, m * F:(m + 1) * F])
            nc.sync.dma_start(out=mut[:, 0:F], in_=mu_d[:, 0:F])
            nc.sync.dma_start(out=mut[:, F:2 * F + 256], in_=mu_d[:, F:2 * F + 256])
            lp_row = mut[0:1, 2 * F:2 * F + 256]

            # constants + PE warmup during the DMA window
            ones_col = sb.tile([128, 1], F16, tag="onc")
            ones_row = sb.tile([1, 128], F16, tag="onr")
            dummy = sb.tile([128, 128], F16, tag="dmy")
            nc.gpsimd.memset(ones_col[:], 1.0)
            nc.gpsimd.memset(ones_row[:], 1.0)
            nc.gpsimd.memset(dummy[:], 0.5)
            tw = sb.tile([1, 1], F32, tag="tw")
            tw2 = sb.tile([1, 1], F32, tag="tw2")
            nc.gpsimd.memset(tw[:], 0.0)
            nc.scalar.activation(tw2[:], tw[:], AF.Exp)  # preload exp table
            wp = po.tile([128, 128], F32, tag="wp")
            for i in range(NWARM):
                nc.tensor.matmul(wp[:], dummy[:], dummy[:], start=True, stop=True)

            # ---- W prep + x^2 (per half h: cols h*1024..(h+1)*1024) ----
            invt = sb.tile([128, 2 * F], F16, tag="invt")
            wqt = sb.tile([128, 2 * F], F16, tag="wqt")
            wct = sb.tile([128, 2 * F], F16, tag="wct")
            x2t = sb.tile([128, 2 * F], F16, tag="x2t")
            m2it = sb.tile([128, 2 * F], F16, tag="m2it")
            tt = sb.tile([128, 2 * F], F16, tag="tt")
            hs = [slice(0, F), slice(F, 2 * F)]
            for h in range(2):
                nc.scalar.activation(invt[:, hs[h]], lvt[:, hs[h]], AF.Exp, scale=-1.0)
                nc.vector.tensor_scalar_mul(wqt[:, hs[h]], invt[:, hs[h]], -0.5)
            nc.scalar.square(x2t[:, hs[0]], xt[:, hs[0]])
            nc.vector.tensor_mul(x2t[:, hs[1]], xt[:, hs[1]], xt[:, hs[1]])
            for h in range(2):
                nc.vector.tensor_mul(wct[:, hs[h]], mut[:, hs[h]], invt[:, hs[h]])
                nc.vector.tensor_mul(m2it[:, hs[h]], mut[:, hs[h]], wct[:, hs[h]])
                nc.vector.tensor_add(tt[:, hs[h]], lvt[:, hs[h]], m2it[:, hs[h]])

            # ---- GEMMs: out[b,c] accumulated in 2 b-half PSUM tiles ----
            x3 = xt[:].rearrange("p (k n) -> p k n", k=KT)
            x23 = x2t[:].rearrange("p (k n) -> p k n", k=KT)
            wq3 = wqt[:].rearrange("p (k n) -> p k n", k=KT)
            wc3 = wct[:].rearrange("p (k n) -> p k n", k=KT)
            t3 = tt[:].rearrange("p (k n) -> p k n", k=KT)
            pg = [po.tile([128, C], F32, tag=f"pg{bh}", name=f"pg{bh}") for bh in range(2)]
            step = [0, 0]
            for A3, W3 in ((x23, wq3), (x3, wc3)):
                for k in range(KT):
                    for bh in range(2):
                        nc.tensor.matmul(
                            pg[bh][:],
                            A3[:, k, bh * 128:(bh + 1) * 128],
                            W3[:, k, :],
                            start=(step[bh] == 0),
                            stop=False,
                            skip_group_check=True,
                        )
                        step[bh] += 1

            # ---- const_c: s = sum_f t; const = lp - 0.5*s - 0.5*F*log2pi ----
            s_ps = po.tile([1, C], F32, tag="sps")
            for k in range(KT):
                nc.tensor.matmul(
                    s_ps[:], ones_col[:], t3[:, k, :],
                    start=(k == 0), stop=(k == KT - 1), skip_group_check=True,
                )
            const_row = sb.tile([1, C], F16, tag="crow")
            cf = sb.tile([1, C], F32, tag="cf")
            nc.vector.tensor_scalar(cf[:], s_ps[:], -0.5, -0.5 * F * LOG_2PI, OP.mult, OP.add)
            nc.vector.tensor_add(const_row[:], cf[:], lp_row)
            for bh in range(2):
                nc.tensor.matmul(
                    pg[bh][:], ones_row[:], const_row[:],
                    start=False, stop=True, skip_group_check=True,
                )

            # ---- copy out + DMA ----
            out_sb = sb.tile([128, 2 * BSH], F16, tag="osb")
            nc.vector.tensor_copy(out_sb[:, 0:BSH], pg[0][:])
            nc.scalar.copy(out=out_sb[:, BSH:2 * BSH], in_=pg[1][:])
            nc.sync.dma_start(out=out_d[:, :], in_=out_sb[:])

    nc.compile()
    return nc


def get_nc():
    if "nc" not in _CACHE:
        _CACHE["nc"] = _build()
    return _CACHE["nc"]


def _pack_fmajor(aT):
    # (F=1024, n) f-major -> SBUF-packed (128, 8*n): cols k*n..(k+1)*n = rows
    # k*128..(k+1)*128
    Fdim, n = aT.shape
    k = Fdim // 128
    return np.ascontiguousarray(
        aT.reshape(k, 128, n).transpose(1, 0, 2).reshape(128, k * n)
    )


def make_in_maps(x, mu, log_var, log_pi):
    x16 = np.asarray(x, dtype=np.float16)
    mu16 = np.asarray(mu, dtype=np.float16)
    lv16 = np.asarray(log_var, dtype=np.float16)
    lp16 = np.asarray(log_pi, dtype=np.float16).reshape(1, C)

    lvt = _pack_fmajor(lv16.T)                      # (128, 2048)
    mut = _pack_fmajor(mu16.T)                      # (128, 2048)
    mut = np.concatenate([mut, np.zeros((128, 256), np.float16)], axis=1)
    mut[0:1, 2 * F:2 * F + 256] = lp16
    mut = np.ascontiguousarray(mut)
    xT = x16.T                                      # (1024, 2048)
    return [
        {"lvt": lvt, "mut": mut,
         "xt": _pack_fmajor(xT[:, c * BSH:(c + 1) * BSH])}
        for c in range(NCORES)
    ]


def unpack_out(res):
    out = np.empty((B, C), dtype=np.float32)
    for c in range(NCORES):
        o = res.results[c]["out"]                   # (128, 512) fp16
        out[c * BSH:c * BSH + 128, :] = o[:, 0:BSH]
        out[c * BSH + 128:(c + 1) * BSH, :] = o[:, BSH:2 * BSH]
    return out


def kernel(x, mu, log_var, log_pi):
    nc = get_nc()
    in_maps = make_in_maps(x, mu, log_var, log_pi)
    res = run_bass_kernel_spmd(nc, in_maps, list(range(NCORES)))
    return unpack_out(res)
